# revision 13
# baseline (speedup 1.0000x reference)
"""Trainium2 Bass kernel for nn_CrossOutLayer.

Math (reference):
    Wx, Wy = W1[:D], W1[D:]
    xp = x @ Wx                      # [B, N1, D]
    yp = y @ Wy                      # [B, N2, D]
    h  = xp[:, :, None, :] + yp[:, None, :, :] + b1
    o  = gelu_exact(h) @ W2 + b2     # [B, N1, N2]

Sharding: 8 cores, each handles one (batch, n1-half) slice -> [256, 512] of
the output. Weights replicated. Inside a core, d lives on partitions
(2 chunks of 128), n2 on the free dim:
  - PE computes ypT = (y @ Wy).T and xpT = (x @ Wx).T once (fp32 matmuls).
  - DVE broadcasts: h[d, n2] = ypT[d, n2] + (xpT[d, n1] + b1[d]) per n1
    (tensor_scalar add with a per-partition scalar, 2x fp32 mode).
  - ACT applies exact Gelu in large batched ops (8 n1 per instruction) --
    this is the roofline engine: B*N1*N2*D/8 = 33.5M lut evals per core.
  - PE reduces over d with W2 as the stationary operand (M=1, N=512,
    float32r so the fp32 matmul streams at 1 cycle/row). Four n1 at a time
    via column tiling (PSUM rows 0/32/64/96 of one bank).
  - DVE copies the PSUM bank to SBUF, DMA stores the 4 rows to HBM.
b2 is added on the host (single scalar).
"""

import os

import numpy as np

B, N1, N2, D = 4, 512, 512, 256
NCORES = 8
NH = N1 * B // NCORES  # 256 n1 rows per core
G = 8                  # n1 values batched per ACT instruction
NGROUPS = NH // G      # 32
P = 128                # partitions / d-chunk size

_BUILT = {}


def _build_nc():
    import concourse.mybir as mybir
    from concourse import bacc
    from concourse.tile import TileContext
    from concourse.bass import ts, ds

    f32 = mybir.dt.float32
    bf16 = mybir.dt.bfloat16
    # XKERNEL_ACT exists only so tests can simulate with an activation that
    # CoreSim implements (e.g. Tanh); production default is exact Gelu.
    GELU = getattr(mybir.ActivationFunctionType,
                   os.environ.get("XKERNEL_ACT", "Gelu"))

    nc = bacc.Bacc("TRN2", target_bir_lowering=False, debug=False)

    xT = nc.dram_tensor("xT", [D, NH], f32, kind="ExternalInput")
    yT = nc.dram_tensor("yT", [D, N2], f32, kind="ExternalInput")
    W1 = nc.dram_tensor("W1", [2 * D, D], f32, kind="ExternalInput")
    b1t = nc.dram_tensor("b1t", [P, 2], f32, kind="ExternalInput")
    w2t = nc.dram_tensor("w2t", [P, 2], f32, kind="ExternalInput")
    out = nc.dram_tensor("out", [NH, N2], f32, kind="ExternalOutput")

    with TileContext(nc) as tc:
        with (
            tc.tile_pool(name="const", bufs=1) as cpool,
            tc.tile_pool(name="hpool", bufs=2) as hpool,
            tc.tile_pool(name="gpool", bufs=2) as gpool,
            tc.tile_pool(name="stage", bufs=3) as spool,
            tc.tile_pool(name="ps_pre", bufs=1, space="PSUM") as pre_ps,
            tc.tile_pool(name="ps_out", bufs=4, space="PSUM") as out_ps,
        ):
            # ---- load inputs ----
            w1s = []
            for j in range(4):
                t = cpool.tile([P, D], f32, tag=f"w1s{j}", name=f"w1s{j}")
                nc.sync.dma_start(out=t[:], in_=W1[ts(j, P), :])
                w1s.append(t)
            xts = []
            for k in range(2):
                t = cpool.tile([P, NH], f32, tag=f"xts{k}", name=f"xts{k}")
                nc.sync.dma_start(out=t[:], in_=xT[ts(k, P), :])
                xts.append(t)
            yts = []
            for k in range(2):
                t = cpool.tile([P, N2], f32, tag=f"yts{k}", name=f"yts{k}")
                nc.sync.dma_start(out=t[:], in_=yT[ts(k, P), :])
                yts.append(t)
            b1tile = cpool.tile([P, 2], f32, tag="b1tile", name="b1tile")
            nc.sync.dma_start(out=b1tile[:], in_=b1t[:])
            w2tile = cpool.tile([P, 2], f32, tag="w2tile", name="w2tile")
            nc.sync.dma_start(out=w2tile[:], in_=w2t[:])
            # bf16 copy of W2 for the fast (1 cycle/row) reduction matmuls;
            # fp32r can't do M=1 / col-tiled output (dst partition must be 0),
            # so the d-reduction runs in bf16 (PSUM still accumulates fp32)
            w2b = cpool.tile([P, 2], bf16, tag="w2b", name="w2b")
            nc.vector.tensor_copy(w2b[:], w2tile[:])
            # zero row used to TensorE-initialize output PSUM banks (K=1
            # matmul with zero operands writes 0 to all 128x512 elements)
            zrow = cpool.tile([1, N2], bf16, tag="zrow", name="zrow")
            nc.vector.memset(zrow[:], 0.0)

            # ---- precompute ypT (d' x n2) and xpbT = xpT + b1 (d' x n1) ----
            ypt, xpbt = [], []
            for c in range(2):
                psy = pre_ps.tile([P, N2], f32, tag="psy", name=f"psy{c}")
                nc.tensor.matmul(psy[:], lhsT=w1s[2][:, ts(c, P)], rhs=yts[0][:],
                                 start=True, stop=False)
                nc.tensor.matmul(psy[:], lhsT=w1s[3][:, ts(c, P)], rhs=yts[1][:],
                                 start=False, stop=True)
                yp_c = cpool.tile([P, N2], f32, tag=f"ypt{c}", name=f"ypt{c}")
                nc.vector.tensor_copy(yp_c[:], psy[:])
                ypt.append(yp_c)

                psx = pre_ps.tile([P, NH], f32, tag="psx", name=f"psx{c}")
                nc.tensor.matmul(psx[:], lhsT=w1s[0][:, ts(c, P)], rhs=xts[0][:],
                                 start=True, stop=False)
                nc.tensor.matmul(psx[:], lhsT=w1s[1][:, ts(c, P)], rhs=xts[1][:],
                                 start=False, stop=True)
                xp_c = cpool.tile([P, NH], f32, tag=f"xpbt{c}", name=f"xpbt{c}")
                nc.vector.tensor_scalar_add(xp_c[:], psx[:], b1tile[:, c:c + 1])
                xpbt.append(xp_c)

            # ---- main loop: groups of G n1 values ----
            for gi in range(NGROUPS):
                hs = []
                for c in range(2):
                    h = hpool.tile([P, G * N2], f32, tag=f"h{c}", name=f"h{c}_{gi}")
                    for i in range(G):
                        n1 = gi * G + i
                        nc.vector.tensor_scalar_add(
                            h[:, ts(i, N2)], ypt[c][:], xpbt[c][:, n1:n1 + 1])
                    hs.append(h)
                gs = []
                for c in range(2):
                    g = gpool.tile([P, G * N2], bf16, tag=f"g{c}", name=f"g{c}_{gi}")
                    nc.scalar.activation(g[:], hs[c][:], GELU)
                    gs.append(g)
                for q in range(2):  # 2 quads of 4 n1
                    pso = out_ps.tile([P, N2], f32, tag="pso", name=f"pso{gi}_{q}")
                    nc.tensor.matmul(
                        pso[:], lhsT=zrow[0:1, 0:P], rhs=zrow[0:1, :],
                        start=True, stop=True)
                    for j in range(4):
                        i = q * 4 + j
                        nc.tensor.matmul(
                            pso[ds(32 * j, 1), :],
                            lhsT=w2b[:, 0:1],
                            rhs=gs[0][:, ts(i, N2)],
                            start=True, stop=False, tile_position=(0, 32 * j))
                        nc.tensor.matmul(
                            pso[ds(32 * j, 1), :],
                            lhsT=w2b[:, 1:2],
                            rhs=gs[1][:, ts(i, N2)],
                            start=False, stop=True, tile_position=(0, 32 * j))
                    stage = spool.tile([P, N2], f32, tag="stage",
                                       name=f"stage{gi}_{q}")
                    nc.vector.tensor_copy(stage[0:97, :], pso[0:97, :])
                    src = stage[:].rearrange("(a b) n -> a b n", b=32)[:, 0, :]
                    nc.sync.dma_start(out=out[ds(gi * G + q * 4, 4), :], in_=src)
    nc.compile()
    return nc


def _get_nc():
    if "nc" not in _BUILT:
        _BUILT["nc"] = _build_nc()
    return _BUILT["nc"]


def _make_in_maps(x, y, W1, b1, W2):
    x = np.ascontiguousarray(np.asarray(x, dtype=np.float32))
    y = np.ascontiguousarray(np.asarray(y, dtype=np.float32))
    W1 = np.ascontiguousarray(np.asarray(W1, dtype=np.float32))
    b1 = np.asarray(b1, dtype=np.float32)
    W2 = np.asarray(W2, dtype=np.float32)
    b1t = np.ascontiguousarray(b1.reshape(2, P).T)
    w2t = np.ascontiguousarray(W2.reshape(2, P).T)
    in_maps = []
    for core in range(NCORES):
        b, half = core // 2, core % 2
        in_maps.append({
            "xT": np.ascontiguousarray(x[b, half * NH:(half + 1) * NH, :].T),
            "yT": np.ascontiguousarray(y[b].T),
            "W1": W1,
            "b1t": b1t,
            "w2t": w2t,
        })
    return in_maps


def _run(x, y, W1, b1, W2, b2, trace=False, **spmd_kwargs):
    from concourse.bass_utils import run_bass_kernel_spmd

    nc = _get_nc()
    in_maps = _make_in_maps(x, y, W1, b1, W2)
    res = run_bass_kernel_spmd(nc, in_maps, list(range(NCORES)), trace=trace,
                               **spmd_kwargs)
    out = np.empty((B, N1, N2), dtype=np.float32)
    for core in range(NCORES):
        b, half = core // 2, core % 2
        out[b, half * NH:(half + 1) * NH, :] = res.results[core]["out"]
    out += np.float32(np.asarray(b2, dtype=np.float32).reshape(-1)[0])
    return out, res


def kernel(x, y, W1, b1, W2, b2):
    out, _ = _run(x, y, W1, b1, W2, b2, trace=False)
    return out


# revision 14
# speedup vs baseline: 1.0409x; 1.0409x over previous
"""Trainium2 Bass kernel for nn_CrossOutLayer.

Math (reference):
    Wx, Wy = W1[:D], W1[D:]
    xp = x @ Wx                      # [B, N1, D]
    yp = y @ Wy                      # [B, N2, D]
    h  = xp[:, :, None, :] + yp[:, None, :, :] + b1
    o  = gelu_exact(h) @ W2 + b2     # [B, N1, N2]

Sharding: 8 cores, each handles one (batch, n1-half) slice -> [256, 512] of
the output. Weights replicated. Inside a core, d lives on partitions
(2 chunks of 128), n2 on the free dim:
  - PE computes ypT = (y @ Wy).T and xpT = (x @ Wx).T once (fp32 matmuls).
  - DVE broadcasts: h[d, n2] = ypT[d, n2] + (xpT[d, n1] + b1[d]) per n1
    (tensor_scalar add with a per-partition scalar, 2x fp32 mode).
  - ACT applies exact Gelu in large batched ops (8 n1 per instruction) --
    this is the roofline engine: B*N1*N2*D/8 = 33.5M lut evals per core.
  - PE reduces over d with W2 as the stationary operand (M=1, N=512,
    float32r so the fp32 matmul streams at 1 cycle/row). Four n1 at a time
    via column tiling (PSUM rows 0/32/64/96 of one bank).
  - DVE copies the PSUM bank to SBUF, DMA stores the 4 rows to HBM.
b2 is added on the host (single scalar).
"""

import os

import numpy as np

B, N1, N2, D = 4, 512, 512, 256
NCORES = 8
NH = N1 * B // NCORES  # 256 n1 rows per core
G = 16                 # n1 values batched per ACT instruction
NGROUPS = NH // G      # 32
P = 128                # partitions / d-chunk size

_BUILT = {}


def _build_nc():
    import concourse.mybir as mybir
    from concourse import bacc
    from concourse.tile import TileContext
    from concourse.bass import ts, ds

    f32 = mybir.dt.float32
    bf16 = mybir.dt.bfloat16
    # XKERNEL_ACT exists only so tests can simulate with an activation that
    # CoreSim implements (e.g. Tanh); production default is exact Gelu.
    GELU = getattr(mybir.ActivationFunctionType,
                   os.environ.get("XKERNEL_ACT", "Gelu"))
    SEED_PSUM = os.environ.get("XKERNEL_SEED", "0") == "1"

    nc = bacc.Bacc("TRN2", target_bir_lowering=False, debug=False)

    xT = nc.dram_tensor("xT", [D, NH], f32, kind="ExternalInput")
    yT = nc.dram_tensor("yT", [D, N2], f32, kind="ExternalInput")
    W1 = nc.dram_tensor("W1", [2 * D, D], f32, kind="ExternalInput")
    b1t = nc.dram_tensor("b1t", [P, 2], f32, kind="ExternalInput")
    w2t = nc.dram_tensor("w2t", [P, 2], f32, kind="ExternalInput")
    out = nc.dram_tensor("out", [NH, N2], f32, kind="ExternalOutput")

    with TileContext(nc) as tc:
        with (
            tc.tile_pool(name="const", bufs=1) as cpool,
            tc.tile_pool(name="hpool", bufs=2) as hpool,
            tc.tile_pool(name="gpool", bufs=2) as gpool,
            tc.tile_pool(name="stage", bufs=3) as spool,
            tc.tile_pool(name="ps_pre", bufs=1, space="PSUM") as pre_ps,
            tc.tile_pool(name="ps_out", bufs=4, space="PSUM") as out_ps,
        ):
            # ---- load inputs ----
            w1s = []
            for j in range(4):
                t = cpool.tile([P, D], f32, tag=f"w1s{j}", name=f"w1s{j}")
                nc.sync.dma_start(out=t[:], in_=W1[ts(j, P), :])
                w1s.append(t)
            xts = []
            for k in range(2):
                t = cpool.tile([P, NH], f32, tag=f"xts{k}", name=f"xts{k}")
                nc.sync.dma_start(out=t[:], in_=xT[ts(k, P), :])
                xts.append(t)
            yts = []
            for k in range(2):
                t = cpool.tile([P, N2], f32, tag=f"yts{k}", name=f"yts{k}")
                nc.sync.dma_start(out=t[:], in_=yT[ts(k, P), :])
                yts.append(t)
            b1tile = cpool.tile([P, 2], f32, tag="b1tile", name="b1tile")
            nc.sync.dma_start(out=b1tile[:], in_=b1t[:])
            w2tile = cpool.tile([P, 2], f32, tag="w2tile", name="w2tile")
            nc.sync.dma_start(out=w2tile[:], in_=w2t[:])
            # bf16 copy of W2 for the fast (1 cycle/row) reduction matmuls;
            # fp32r can't do M=1 / col-tiled output (dst partition must be 0),
            # so the d-reduction runs in bf16 (PSUM still accumulates fp32)
            w2b = cpool.tile([P, 2], bf16, tag="w2b", name="w2b")
            nc.vector.tensor_copy(w2b[:], w2tile[:])
            # zero row used to TensorE-initialize output PSUM banks (K=1
            # matmul with zero operands writes 0 to all 128x512 elements)
            zrow = cpool.tile([1, N2], bf16, tag="zrow", name="zrow")
            nc.vector.memset(zrow[:], 0.0)

            # ---- precompute ypT (d' x n2) and xpbT = xpT + b1 (d' x n1) ----
            ypt, xpbt = [], []
            for c in range(2):
                psy = pre_ps.tile([P, N2], f32, tag="psy", name=f"psy{c}")
                nc.tensor.matmul(psy[:], lhsT=w1s[2][:, ts(c, P)], rhs=yts[0][:],
                                 start=True, stop=False)
                nc.tensor.matmul(psy[:], lhsT=w1s[3][:, ts(c, P)], rhs=yts[1][:],
                                 start=False, stop=True)
                yp_c = cpool.tile([P, N2], bf16, tag=f"ypt{c}", name=f"ypt{c}")
                nc.vector.tensor_copy(yp_c[:], psy[:])
                ypt.append(yp_c)

                psx = pre_ps.tile([P, NH], f32, tag="psx", name=f"psx{c}")
                nc.tensor.matmul(psx[:], lhsT=w1s[0][:, ts(c, P)], rhs=xts[0][:],
                                 start=True, stop=False)
                nc.tensor.matmul(psx[:], lhsT=w1s[1][:, ts(c, P)], rhs=xts[1][:],
                                 start=False, stop=True)
                xp_c = cpool.tile([P, NH], f32, tag=f"xpbt{c}", name=f"xpbt{c}")
                nc.vector.tensor_scalar_add(xp_c[:], psx[:], b1tile[:, c:c + 1])
                xpbt.append(xp_c)

            # ---- main loop: groups of G n1 values ----
            for gi in range(NGROUPS):
                hs = []
                for c in range(2):
                    h = hpool.tile([P, G * N2], bf16, tag=f"h{c}", name=f"h{c}_{gi}")
                    for i in range(G):
                        n1 = gi * G + i
                        nc.vector.tensor_scalar_add(
                            h[:, ts(i, N2)], ypt[c][:], xpbt[c][:, n1:n1 + 1])
                    hs.append(h)
                gs = []
                for c in range(2):
                    g = gpool.tile([P, G * N2], bf16, tag=f"g{c}", name=f"g{c}_{gi}")
                    nc.scalar.activation(g[:], hs[c][:], GELU)
                    gs.append(g)
                for q in range(G // 4):  # quads of 4 n1
                    pso = out_ps.tile([P, N2], f32, tag="pso", name=f"pso{gi}_{q}")
                    if SEED_PSUM:
                        # TensorE-initialize all 128 rows so CoreSim sees no
                        # uninitialized PSUM reads; on HW the garbage rows
                        # are copied and discarded, so skip the extra matmul
                        nc.tensor.matmul(
                            pso[:], lhsT=zrow[0:1, 0:P], rhs=zrow[0:1, :],
                            start=True, stop=True)
                    for j in range(4):
                        i = q * 4 + j
                        nc.tensor.matmul(
                            pso[ds(32 * j, 1), :],
                            lhsT=w2b[:, 0:1],
                            rhs=gs[0][:, ts(i, N2)],
                            start=True, stop=False, tile_position=(0, 32 * j))
                        nc.tensor.matmul(
                            pso[ds(32 * j, 1), :],
                            lhsT=w2b[:, 1:2],
                            rhs=gs[1][:, ts(i, N2)],
                            start=False, stop=True, tile_position=(0, 32 * j))
                    stage = spool.tile([P, N2], f32, tag="stage",
                                       name=f"stage{gi}_{q}")
                    nc.vector.tensor_copy(stage[0:97, :], pso[0:97, :])
                    src = stage[:].rearrange("(a b) n -> a b n", b=32)[:, 0, :]
                    nc.sync.dma_start(out=out[ds(gi * G + q * 4, 4), :], in_=src)
    nc.compile()
    return nc


def _get_nc():
    if "nc" not in _BUILT:
        _BUILT["nc"] = _build_nc()
    return _BUILT["nc"]


def _make_in_maps(x, y, W1, b1, W2):
    x = np.ascontiguousarray(np.asarray(x, dtype=np.float32))
    y = np.ascontiguousarray(np.asarray(y, dtype=np.float32))
    W1 = np.ascontiguousarray(np.asarray(W1, dtype=np.float32))
    b1 = np.asarray(b1, dtype=np.float32)
    W2 = np.asarray(W2, dtype=np.float32)
    b1t = np.ascontiguousarray(b1.reshape(2, P).T)
    w2t = np.ascontiguousarray(W2.reshape(2, P).T)
    in_maps = []
    for core in range(NCORES):
        b, half = core // 2, core % 2
        in_maps.append({
            "xT": np.ascontiguousarray(x[b, half * NH:(half + 1) * NH, :].T),
            "yT": np.ascontiguousarray(y[b].T),
            "W1": W1,
            "b1t": b1t,
            "w2t": w2t,
        })
    return in_maps


def _run(x, y, W1, b1, W2, b2, trace=False, **spmd_kwargs):
    from concourse.bass_utils import run_bass_kernel_spmd

    nc = _get_nc()
    in_maps = _make_in_maps(x, y, W1, b1, W2)
    res = run_bass_kernel_spmd(nc, in_maps, list(range(NCORES)), trace=trace,
                               **spmd_kwargs)
    out = np.empty((B, N1, N2), dtype=np.float32)
    for core in range(NCORES):
        b, half = core // 2, core % 2
        out[b, half * NH:(half + 1) * NH, :] = res.results[core]["out"]
    out += np.float32(np.asarray(b2, dtype=np.float32).reshape(-1)[0])
    return out, res


def kernel(x, y, W1, b1, W2, b2):
    out, _ = _run(x, y, W1, b1, W2, b2, trace=False)
    return out
